# revision 8
# baseline (speedup 1.0000x reference)
# kernel.py — Trainium2 Bass kernel for nn_ActivationBuffer (ring-buffer masked
# scatter of activations into a cache, data-parallel over 8 NeuronCores).
#
# Self-contained: only imports installed packages (numpy/jax/concourse).
#
# Semantics reproduced per row d (see the reference module):
#   offsets  = cumsum(mask) - 1
#   acc      = offsets[-1] + 1
#   new_n_valid = min(n_valid + acc, M)
#   new_index   = (index + acc) % M
#   new_cache   = cache;  for each kept token j (in order):
#                   new_cache[(index + j) % M] = acts[t_j]
#                 and if mask[0] == 0: new_cache[(index - 1) % M] = 0
#   (the zeroed slot comes from the reference's .at[i].set(0) over slots
#    touched by leading masked-out tokens, whose offset is -1)
#
# Strategy: the kept tokens' destinations are data-dependent, so the kernel
# computes the per-token destination table on-device (mask cumsum via a
# free-axis scan + a triangular matmul across partitions), then uses one
# indirect DMA gather (loads only kept activation rows; masked rows are
# skipped via the bounds check) and one indirect DMA scatter (writes them to
# their ring slots). All other cache slots are preserved by donating the
# input cache shard as the initial contents of the output buffer — the same
# donated-buffer mechanism the stock axon runner uses with zero buffers.

import numpy as np

D, T, M, DIM = 8, 4096, 32768, 768
P = 128           # SBUF partitions
F = T // P        # 32 tokens per partition row

_STATE = {}


def _build_nc():
    from concourse import bacc, bass, mybir, tile
    from concourse.masks import make_upper_triangular

    dt = mybir.dt
    A = mybir.AluOpType

    nc = bacc.Bacc(
        "TRN2",
        target_bir_lowering=False,
        debug=False,
        enable_partition_id=False,
        enable_asserts=False,
    )

    acts = nc.dram_tensor("activations", [T, DIM], dt.float32, kind="ExternalInput")
    mask = nc.dram_tensor("mask", [T], dt.uint8, kind="ExternalInput")
    n_valid = nc.dram_tensor("n_valid", [1], dt.int32, kind="ExternalInput")
    index = nc.dram_tensor("index", [1], dt.int32, kind="ExternalInput")
    new_cache = nc.dram_tensor("new_cache", [M, DIM], dt.float32, kind="ExternalOutput")
    new_n_valid = nc.dram_tensor("new_n_valid", [1], dt.int32, kind="ExternalOutput")
    new_index = nc.dram_tensor("new_index", [1], dt.int32, kind="ExternalOutput")

    with tile.TileContext(nc) as tc:
        with (
            tc.tile_pool(name="sb", bufs=1) as sb,
            tc.tile_pool(name="ps", bufs=1, space="PSUM") as ps,
        ):
            # ---- activation loads: chunk 0 on the ACT HWDGE ring first (it
            # gates the first scatters), the rest split across both rings
            LC = 8  # load-chunk: tokens per partition per regular DMA
            acts_sb = sb.tile([P, F * DIM], dt.float32)
            acts_pf = acts.ap().rearrange("(p j) d -> p (j d)", p=P)
            for c, eng_name in ((0, "scalar"), (1, "sync"), (2, "scalar"),
                                (3, "sync")):
                es = slice(c * LC * DIM, (c + 1) * LC * DIM)
                getattr(nc, eng_name).dma_start(out=acts_sb[:, es],
                                                in_=acts_pf[:, es])

            # ---- load mask [T] as [P, F] (token t = p*F + f) and the scalars
            m_u8 = sb.tile([P, F], dt.uint8)
            nc.sync.dma_start(out=m_u8[:], in_=mask.ap().rearrange("(p f) -> p f", p=P))
            m_f = sb.tile([P, F], dt.float32)
            nc.vector.tensor_copy(out=m_f[:], in_=m_u8[:])

            idx_i = sb.tile([1, 1], dt.int32)
            nc.sync.dma_start(out=idx_i[:], in_=index.ap()[None, :])
            idx_f = sb.tile([1, 1], dt.float32)
            nc.vector.tensor_copy(out=idx_f[:], in_=idx_i[:])
            nv_i = sb.tile([1, 1], dt.int32)
            nc.sync.dma_start(out=nv_i[:], in_=n_valid.ap()[None, :])
            nv_f = sb.tile([1, 1], dt.float32)
            nc.vector.tensor_copy(out=nv_f[:], in_=nv_i[:])

            # ---- intra-row inclusive cumsum of the mask along the free axis
            zeros_f = sb.tile([P, F], dt.float32)
            nc.vector.memset(zeros_f[:], 0.0)
            C = sb.tile([P, F], dt.float32)
            nc.vector.tensor_tensor_scan(
                out=C[:], data0=m_f[:], data1=zeros_f[:],
                initial=0.0, op0=A.add, op1=A.add,
            )

            # ---- constants for the cross-partition pieces
            stri = sb.tile([P, P], dt.float32)
            make_upper_triangular(nc, stri[:], val=1.0, diag=False)  # stri[k,p]=1 iff k<p
            ones_col = sb.tile([P, 1], dt.float32)
            nc.vector.memset(ones_col[:], 1.0)
            ones_row = sb.tile([1, P], dt.float32)
            nc.vector.memset(ones_row[:], 1.0)

            # ---- E[p] = sum_{k<p} rowtotal[k];  I[p] = index (broadcast)
            e_ps = ps.tile([P, 1], dt.float32)
            nc.tensor.matmul(out=e_ps[:], lhsT=stri[:], rhs=C[:, F - 1:F],
                             start=True, stop=True)
            i_ps = ps.tile([P, 1], dt.float32)
            nc.tensor.matmul(out=i_ps[:], lhsT=ones_row[:], rhs=idx_f[:],
                             start=True, stop=True)
            e_sb = sb.tile([P, 1], dt.float32)
            nc.vector.tensor_copy(out=e_sb[:], in_=e_ps[:])
            B = sb.tile([P, 1], dt.float32)  # index + E[p]
            nc.vector.tensor_tensor(out=B[:], in0=e_sb[:], in1=i_ps[:], op=A.add)

            # ---- acc = total kept tokens, landed on partition 0
            acc_ps = ps.tile([1, 1], dt.float32)
            nc.tensor.matmul(out=acc_ps[:], lhsT=ones_col[:], rhs=C[:, F - 1:F],
                             start=True, stop=True)
            acc = sb.tile([1, 1], dt.float32)
            nc.vector.tensor_copy(out=acc[:], in_=acc_ps[:])

            # ---- destination table: kept -> (index + cumsum - 1) mod M, else M
            # draw = (C - 1) + (index + E[p]), fused
            draw = sb.tile([P, F], dt.float32)
            nc.vector.scalar_tensor_tensor(out=draw[:], in0=C[:], scalar=-1.0,
                                           in1=B[:].to_broadcast([P, F]),
                                           op0=A.add, op1=A.add)
            # gm = (draw >= M) * M, fused
            gm = sb.tile([P, F], dt.float32)
            nc.vector.tensor_scalar(out=gm[:], in0=draw[:], scalar1=float(M),
                                    scalar2=float(M), op0=A.is_ge, op1=A.mult)
            dw = sb.tile([P, F], dt.float32)
            nc.vector.tensor_tensor(out=dw[:], in0=draw[:], in1=gm[:], op=A.subtract)
            # dest = (dw - M) * m + M  -> masked-out lanes get sentinel M
            t2 = sb.tile([P, F], dt.float32)
            nc.vector.scalar_tensor_tensor(out=t2[:], in0=dw[:],
                                           scalar=-float(M), in1=m_f[:],
                                           op0=A.add, op1=A.mult)
            dest_f = sb.tile([P, F], dt.float32)
            nc.vector.tensor_scalar_add(out=dest_f[:], in0=t2[:], scalar1=float(M))
            dest = sb.tile([P, F], dt.int32)
            nc.vector.tensor_copy(out=dest[:], in_=dest_f[:])

            # ---- load all activations into SBUF with fast regular DMAs
            # (token t = p*F + j lives at partition p, free chunk j), then
            # scatter kept rows to their ring slots. The HW indirect DMA
            # consumes ONE offset per partition (each feeding that
            # partition's contiguous block), so each scatter op moves one
            # 768-float row per partition using a [P, 1] slice of the dest
            # table; sentinel entries are skipped via the bounds check.
            scatter_insts = []
            for j in range(F):
                si = nc.gpsimd.indirect_dma_start(
                    out=new_cache.ap(),
                    out_offset=bass.IndirectOffsetOnAxis(ap=dest[:, j:j + 1], axis=0),
                    in_=acts_sb[:, j * DIM:(j + 1) * DIM], in_offset=None,
                    bounds_check=M - 1, oob_is_err=False,
                )
                scatter_insts.append(si)
            # The scatters all declare a write of the full new_cache AP, so
            # Tile chains them WAW (each waits for the previous one's DMA
            # completion, ~2us/op). Their actual row sets are disjoint (kept
            # tokens map to distinct ring slots; sentinels are skipped), so
            # drop the inter-scatter edges and let them issue back-to-back.
            _raw = [s.ins for s in scatter_insts]
            _names = {r.name for r in _raw}
            for r in _raw:
                for dep in list(r.sync_dependency_names()) + \
                        list(r.nosync_dependency_names()):
                    if dep in _names:
                        r.remove_dependency(dep)

            # ---- leading-masked-token edge case: zero slot (index-1) mod M
            zrow = sb.tile([2, DIM], dt.float32)
            nc.vector.memset(zrow[:], 0.0)
            zoff = sb.tile([2, 1], dt.int32)
            nc.vector.memset(zoff[:], M)
            zt = sb.tile([1, 1], dt.float32)
            nc.vector.tensor_scalar_add(out=zt[:], in0=idx_f[:], scalar1=-1.0)
            zneg = sb.tile([1, 1], dt.float32)
            nc.vector.tensor_scalar(out=zneg[:], in0=zt[:], scalar1=0.0,
                                    scalar2=None, op0=A.is_lt)
            zm = sb.tile([1, 1], dt.float32)
            nc.vector.tensor_scalar_mul(out=zm[:], in0=zneg[:], scalar1=float(M))
            zw = sb.tile([1, 1], dt.float32)
            nc.vector.tensor_tensor(out=zw[:], in0=zt[:], in1=zm[:], op=A.add)
            za = sb.tile([1, 1], dt.float32)
            nc.vector.tensor_scalar_add(out=za[:], in0=zw[:], scalar1=-float(M))
            zb = sb.tile([1, 1], dt.float32)
            nc.vector.tensor_tensor(out=zb[:], in0=za[:], in1=m_f[0:1, 0:1], op=A.mult)
            zc = sb.tile([1, 1], dt.float32)
            nc.vector.tensor_tensor(out=zc[:], in0=za[:], in1=zb[:], op=A.subtract)
            zd = sb.tile([1, 1], dt.float32)
            nc.vector.tensor_scalar_add(out=zd[:], in0=zc[:], scalar1=float(M))
            nc.vector.tensor_copy(out=zoff[0:1, 0:1], in_=zd[:])
            nc.gpsimd.indirect_dma_start(
                out=new_cache.ap(),
                out_offset=bass.IndirectOffsetOnAxis(ap=zoff[:, 0:1], axis=0),
                in_=zrow[:], in_offset=None,
                bounds_check=M - 1, oob_is_err=False,
            )

            # ---- scalar outputs: new_index = (index+acc) mod M,
            # ----                 new_n_valid = min(n_valid+acc, M)
            s1 = sb.tile([1, 1], dt.float32)
            nc.vector.tensor_tensor(out=s1[:], in0=idx_f[:], in1=acc[:], op=A.add)
            s2 = sb.tile([1, 1], dt.float32)
            nc.vector.tensor_scalar(out=s2[:], in0=s1[:], scalar1=float(M),
                                    scalar2=None, op0=A.is_ge)
            s3 = sb.tile([1, 1], dt.float32)
            nc.vector.tensor_scalar_mul(out=s3[:], in0=s2[:], scalar1=float(M))
            s4 = sb.tile([1, 1], dt.float32)
            nc.vector.tensor_tensor(out=s4[:], in0=s1[:], in1=s3[:], op=A.subtract)
            ni_o = sb.tile([1, 1], dt.int32)
            nc.vector.tensor_copy(out=ni_o[:], in_=s4[:])
            nc.sync.dma_start(out=new_index.ap()[None, :], in_=ni_o[:])

            v1 = sb.tile([1, 1], dt.float32)
            nc.vector.tensor_tensor(out=v1[:], in0=nv_f[:], in1=acc[:], op=A.add)
            v2 = sb.tile([1, 1], dt.float32)
            nc.vector.tensor_scalar_min(out=v2[:], in0=v1[:], scalar1=float(M))
            nv_o = sb.tile([1, 1], dt.int32)
            nc.vector.tensor_copy(out=nv_o[:], in_=v2[:])
            nc.sync.dma_start(out=new_n_valid.ap()[None, :], in_=nv_o[:])

    nc.finalize()
    return nc


_IN_NAMES = ("activations", "mask", "n_valid", "index")
_OUT_NAMES = ("new_cache", "new_n_valid", "new_index")


def get_nc():
    if "nc" not in _STATE:
        _STATE["nc"] = _build_nc()
    return _STATE["nc"]


def _get_sharded_fn(nc, n_cores):
    """jit(shard_map(bass_exec)) with the output buffers passed in as donated
    inputs — adapted from concourse.bass2jax.run_bass_via_pjrt, except the
    donated buffers carry caller-supplied initial data instead of zeros (the
    NEFF only writes the scattered rows; the rest of new_cache must keep the
    input cache contents)."""
    key = ("fn", n_cores)
    if key in _STATE:
        return _STATE[key]

    import jax
    from jax.experimental.shard_map import shard_map
    from jax.sharding import Mesh, PartitionSpec
    import concourse.mybir as mybir
    from concourse.bass2jax import _bass_exec_p, install_neuronx_cc_hook

    install_neuronx_cc_hook()

    in_names = []
    out_names = []
    out_avals = []
    for alloc in nc.m.functions[0].allocations:
        if not isinstance(alloc, mybir.MemoryLocationSet):
            continue
        name = alloc.memorylocations[0].name
        if alloc.kind == "ExternalInput":
            in_names.append(name)
        elif alloc.kind == "ExternalOutput":
            out_names.append(name)
            out_avals.append(
                jax.core.ShapedArray(tuple(alloc.tensor_shape),
                                     mybir.dt.np(alloc.dtype))
            )
    n_params = len(in_names)
    n_outs = len(out_avals)
    all_in_names = in_names + out_names
    donate = tuple(range(n_params, n_params + n_outs))

    def _body(*args):
        outs = _bass_exec_p.bind(
            *args,
            out_avals=tuple(out_avals),
            in_names=tuple(all_in_names),
            out_names=tuple(out_names),
            lowering_input_output_aliases=(),
            sim_require_finite=True,
            sim_require_nnan=True,
            nc=nc,
        )
        return tuple(outs)

    devices = jax.devices()[:n_cores]
    assert len(devices) == n_cores, f"need {n_cores} devices, have {len(jax.devices())}"
    mesh = Mesh(np.asarray(devices), ("core",))
    spec = PartitionSpec("core")
    sharded = jax.jit(
        shard_map(
            _body, mesh=mesh,
            in_specs=(spec,) * (n_params + n_outs),
            out_specs=(spec,) * n_outs,
            check_rep=False,
        ),
        donate_argnums=donate,
        keep_unused=True,
    )
    _STATE[key] = (sharded, tuple(in_names), tuple(out_names))
    return _STATE[key]


def run_on_cores(global_ins, global_init_outs, n_cores=D):
    """Run the SPMD kernel. `global_ins`/`global_init_outs` are dicts of
    device-count-concatenated arrays (axis 0 splits across cores). Returns a
    dict of global output arrays."""
    nc = get_nc()
    sharded, in_names, out_names = _get_sharded_fn(nc, n_cores)
    args = [np.ascontiguousarray(global_ins[n]) for n in in_names]
    args += [np.ascontiguousarray(global_init_outs[n]) for n in out_names]
    out_arrs = sharded(*args)
    return {n: np.asarray(a) for n, a in zip(out_names, out_arrs)}


def kernel(cache, activations, mask, n_valid, index):
    cache = np.asarray(cache, dtype=np.float32)
    activations = np.asarray(activations, dtype=np.float32)
    mask_u8 = np.asarray(mask).astype(np.uint8)
    n_valid = np.asarray(n_valid, dtype=np.int32)
    index = np.asarray(index, dtype=np.int32)

    global_ins = {
        "activations": activations.reshape(D * T, DIM),
        "mask": mask_u8.reshape(D * T),
        "n_valid": n_valid.reshape(D),
        "index": index.reshape(D),
    }
    global_init_outs = {
        # donated buffer: the input cache — the kernel overwrites only the
        # scattered ring slots
        "new_cache": cache.reshape(D * M, DIM),
        "new_n_valid": np.zeros(D, np.int32),
        "new_index": np.zeros(D, np.int32),
    }
    outs = run_on_cores(global_ins, global_init_outs)
    new_cache = outs["new_cache"].reshape(D, M, DIM)
    new_n_valid = outs["new_n_valid"].reshape(D).astype(np.int32)
    new_index = outs["new_index"].reshape(D).astype(np.int32)
    return new_cache, new_n_valid, new_index


# revision 9
# speedup vs baseline: 1.3530x; 1.3530x over previous
# kernel.py — Trainium2 Bass kernel for nn_ActivationBuffer (ring-buffer masked
# scatter of activations into a cache, data-parallel over 8 NeuronCores).
#
# Self-contained: only imports installed packages (numpy/jax/concourse).
#
# Semantics reproduced per row d (see the reference module):
#   offsets  = cumsum(mask) - 1
#   acc      = offsets[-1] + 1
#   new_n_valid = min(n_valid + acc, M)
#   new_index   = (index + acc) % M
#   new_cache   = cache;  for each kept token j (in order):
#                   new_cache[(index + j) % M] = acts[t_j]
#                 and if mask[0] == 0: new_cache[(index - 1) % M] = 0
#   (the zeroed slot comes from the reference's .at[i].set(0) over slots
#    touched by leading masked-out tokens, whose offset is -1)
#
# Strategy: the kept tokens' destinations are data-dependent, so the kernel
# computes the per-token destination table on-device (mask cumsum via a
# free-axis scan + a triangular matmul across partitions), then uses one
# indirect DMA gather (loads only kept activation rows; masked rows are
# skipped via the bounds check) and one indirect DMA scatter (writes them to
# their ring slots). All other cache slots are preserved by donating the
# input cache shard as the initial contents of the output buffer — the same
# donated-buffer mechanism the stock axon runner uses with zero buffers.

import numpy as np

D, T, M, DIM = 8, 4096, 32768, 768
P = 128           # SBUF partitions
F = T // P        # 32 tokens per partition row

_STATE = {}


def _build_nc():
    from concourse import bacc, bass, mybir, tile
    from concourse.masks import make_upper_triangular

    dt = mybir.dt
    A = mybir.AluOpType

    nc = bacc.Bacc(
        "TRN2",
        target_bir_lowering=False,
        debug=False,
        enable_partition_id=False,
        enable_asserts=False,
    )

    acts = nc.dram_tensor("activations", [T, DIM], dt.float32, kind="ExternalInput")
    mask = nc.dram_tensor("mask", [T], dt.uint8, kind="ExternalInput")
    n_valid = nc.dram_tensor("n_valid", [1], dt.int32, kind="ExternalInput")
    index = nc.dram_tensor("index", [1], dt.int32, kind="ExternalInput")
    new_cache = nc.dram_tensor("new_cache", [M, DIM], dt.float32, kind="ExternalOutput")
    new_n_valid = nc.dram_tensor("new_n_valid", [1], dt.int32, kind="ExternalOutput")
    new_index = nc.dram_tensor("new_index", [1], dt.int32, kind="ExternalOutput")

    with tile.TileContext(nc) as tc:
        with (
            tc.tile_pool(name="sb", bufs=1) as sb,
            tc.tile_pool(name="ps", bufs=1, space="PSUM") as ps,
        ):
            # ---- tiny metadata loads FIRST on the SP ring (the mask gates
            # the whole destination-table chain; it must not queue behind
            # the multi-MB activation loads)
            m_u8 = sb.tile([P, F], dt.uint8)
            nc.sync.dma_start(out=m_u8[:], in_=mask.ap().rearrange("(p f) -> p f", p=P))
            idx_i = sb.tile([1, 1], dt.int32)
            nc.sync.dma_start(out=idx_i[:], in_=index.ap()[None, :])
            nv_i = sb.tile([1, 1], dt.int32)
            nc.sync.dma_start(out=nv_i[:], in_=n_valid.ap()[None, :])

            # ---- activation loads, split across the two HWDGE rings: chunk
            # 0 leads on the (otherwise idle) ACT ring since the first
            # scatters need it
            LC = 8  # load-chunk: tokens per partition per regular DMA
            acts_sb = sb.tile([P, F * DIM], dt.float32)
            acts_pf = acts.ap().rearrange("(p j) d -> p (j d)", p=P)
            for c, eng_name in ((0, "scalar"), (1, "sync"), (2, "scalar"),
                                (3, "sync")):
                es = slice(c * LC * DIM, (c + 1) * LC * DIM)
                getattr(nc, eng_name).dma_start(out=acts_sb[:, es],
                                                in_=acts_pf[:, es])

            m_f = sb.tile([P, F], dt.float32)
            nc.vector.tensor_copy(out=m_f[:], in_=m_u8[:])
            idx_f = sb.tile([1, 1], dt.float32)
            nc.vector.tensor_copy(out=idx_f[:], in_=idx_i[:])
            nv_f = sb.tile([1, 1], dt.float32)
            nc.vector.tensor_copy(out=nv_f[:], in_=nv_i[:])

            # ---- intra-row inclusive cumsum of the mask along the free axis
            zeros_f = sb.tile([P, F], dt.float32)
            nc.vector.memset(zeros_f[:], 0.0)
            C = sb.tile([P, F], dt.float32)
            nc.vector.tensor_tensor_scan(
                out=C[:], data0=m_f[:], data1=zeros_f[:],
                initial=0.0, op0=A.add, op1=A.add,
            )

            # ---- constants for the cross-partition pieces
            stri = sb.tile([P, P], dt.float32)
            make_upper_triangular(nc, stri[:], val=1.0, diag=False)  # stri[k,p]=1 iff k<p
            ones_col = sb.tile([P, 1], dt.float32)
            nc.vector.memset(ones_col[:], 1.0)
            ones_row = sb.tile([1, P], dt.float32)
            nc.vector.memset(ones_row[:], 1.0)

            # ---- E[p] = sum_{k<p} rowtotal[k];  I[p] = index (broadcast)
            e_ps = ps.tile([P, 1], dt.float32)
            nc.tensor.matmul(out=e_ps[:], lhsT=stri[:], rhs=C[:, F - 1:F],
                             start=True, stop=True)
            i_ps = ps.tile([P, 1], dt.float32)
            nc.tensor.matmul(out=i_ps[:], lhsT=ones_row[:], rhs=idx_f[:],
                             start=True, stop=True)
            e_sb = sb.tile([P, 1], dt.float32)
            nc.vector.tensor_copy(out=e_sb[:], in_=e_ps[:])
            B = sb.tile([P, 1], dt.float32)  # index + E[p]
            nc.vector.tensor_tensor(out=B[:], in0=e_sb[:], in1=i_ps[:], op=A.add)

            # ---- acc = total kept tokens, landed on partition 0
            acc_ps = ps.tile([1, 1], dt.float32)
            nc.tensor.matmul(out=acc_ps[:], lhsT=ones_col[:], rhs=C[:, F - 1:F],
                             start=True, stop=True)
            acc = sb.tile([1, 1], dt.float32)
            nc.vector.tensor_copy(out=acc[:], in_=acc_ps[:])

            # ---- destination table: kept -> (index + cumsum - 1) mod M, else M
            # draw = (C - 1) + (index + E[p]), fused
            draw = sb.tile([P, F], dt.float32)
            nc.vector.scalar_tensor_tensor(out=draw[:], in0=C[:], scalar=-1.0,
                                           in1=B[:].to_broadcast([P, F]),
                                           op0=A.add, op1=A.add)
            # gm = (draw >= M) * M, fused
            gm = sb.tile([P, F], dt.float32)
            nc.vector.tensor_scalar(out=gm[:], in0=draw[:], scalar1=float(M),
                                    scalar2=float(M), op0=A.is_ge, op1=A.mult)
            dw = sb.tile([P, F], dt.float32)
            nc.vector.tensor_tensor(out=dw[:], in0=draw[:], in1=gm[:], op=A.subtract)
            # dest = (dw - M) * m + M  -> masked-out lanes get sentinel M
            t2 = sb.tile([P, F], dt.float32)
            nc.vector.scalar_tensor_tensor(out=t2[:], in0=dw[:],
                                           scalar=-float(M), in1=m_f[:],
                                           op0=A.add, op1=A.mult)
            dest_f = sb.tile([P, F], dt.float32)
            nc.vector.tensor_scalar_add(out=dest_f[:], in0=t2[:], scalar1=float(M))
            dest = sb.tile([P, F], dt.int32)
            nc.vector.tensor_copy(out=dest[:], in_=dest_f[:])

            # ---- load all activations into SBUF with fast regular DMAs
            # (token t = p*F + j lives at partition p, free chunk j), then
            # scatter kept rows to their ring slots. The HW indirect DMA
            # consumes ONE offset per partition (each feeding that
            # partition's contiguous block), so each scatter op moves one
            # 768-float row per partition using a [P, 1] slice of the dest
            # table; sentinel entries are skipped via the bounds check.
            scatter_insts = []
            for j in range(F):
                si = nc.gpsimd.indirect_dma_start(
                    out=new_cache.ap(),
                    out_offset=bass.IndirectOffsetOnAxis(ap=dest[:, j:j + 1], axis=0),
                    in_=acts_sb[:, j * DIM:(j + 1) * DIM], in_offset=None,
                    bounds_check=M - 1, oob_is_err=False,
                )
                scatter_insts.append(si)
            # The scatters all declare a write of the full new_cache AP, so
            # Tile chains them WAW (each waits for the previous one's DMA
            # completion, ~2us/op). Their actual row sets are disjoint (kept
            # tokens map to distinct ring slots; sentinels are skipped), so
            # drop the inter-scatter edges and let them issue back-to-back.
            _raw = [s.ins for s in scatter_insts]
            _names = {r.name for r in _raw}
            for r in _raw:
                for dep in list(r.sync_dependency_names()) + \
                        list(r.nosync_dependency_names()):
                    if dep in _names:
                        r.remove_dependency(dep)

            # ---- leading-masked-token edge case: zero slot (index-1) mod M
            zrow = sb.tile([2, DIM], dt.float32)
            nc.vector.memset(zrow[:], 0.0)
            zoff = sb.tile([2, 1], dt.int32)
            nc.vector.memset(zoff[:], M)
            zt = sb.tile([1, 1], dt.float32)
            nc.vector.tensor_scalar_add(out=zt[:], in0=idx_f[:], scalar1=-1.0)
            zneg = sb.tile([1, 1], dt.float32)
            nc.vector.tensor_scalar(out=zneg[:], in0=zt[:], scalar1=0.0,
                                    scalar2=None, op0=A.is_lt)
            zm = sb.tile([1, 1], dt.float32)
            nc.vector.tensor_scalar_mul(out=zm[:], in0=zneg[:], scalar1=float(M))
            zw = sb.tile([1, 1], dt.float32)
            nc.vector.tensor_tensor(out=zw[:], in0=zt[:], in1=zm[:], op=A.add)
            za = sb.tile([1, 1], dt.float32)
            nc.vector.tensor_scalar_add(out=za[:], in0=zw[:], scalar1=-float(M))
            zb = sb.tile([1, 1], dt.float32)
            nc.vector.tensor_tensor(out=zb[:], in0=za[:], in1=m_f[0:1, 0:1], op=A.mult)
            zc = sb.tile([1, 1], dt.float32)
            nc.vector.tensor_tensor(out=zc[:], in0=za[:], in1=zb[:], op=A.subtract)
            zd = sb.tile([1, 1], dt.float32)
            nc.vector.tensor_scalar_add(out=zd[:], in0=zc[:], scalar1=float(M))
            nc.vector.tensor_copy(out=zoff[0:1, 0:1], in_=zd[:])
            nc.gpsimd.indirect_dma_start(
                out=new_cache.ap(),
                out_offset=bass.IndirectOffsetOnAxis(ap=zoff[:, 0:1], axis=0),
                in_=zrow[:], in_offset=None,
                bounds_check=M - 1, oob_is_err=False,
            )

            # ---- scalar outputs: new_index = (index+acc) mod M,
            # ----                 new_n_valid = min(n_valid+acc, M)
            s1 = sb.tile([1, 1], dt.float32)
            nc.vector.tensor_tensor(out=s1[:], in0=idx_f[:], in1=acc[:], op=A.add)
            s2 = sb.tile([1, 1], dt.float32)
            nc.vector.tensor_scalar(out=s2[:], in0=s1[:], scalar1=float(M),
                                    scalar2=None, op0=A.is_ge)
            s3 = sb.tile([1, 1], dt.float32)
            nc.vector.tensor_scalar_mul(out=s3[:], in0=s2[:], scalar1=float(M))
            s4 = sb.tile([1, 1], dt.float32)
            nc.vector.tensor_tensor(out=s4[:], in0=s1[:], in1=s3[:], op=A.subtract)
            ni_o = sb.tile([1, 1], dt.int32)
            nc.vector.tensor_copy(out=ni_o[:], in_=s4[:])
            nc.sync.dma_start(out=new_index.ap()[None, :], in_=ni_o[:])

            v1 = sb.tile([1, 1], dt.float32)
            nc.vector.tensor_tensor(out=v1[:], in0=nv_f[:], in1=acc[:], op=A.add)
            v2 = sb.tile([1, 1], dt.float32)
            nc.vector.tensor_scalar_min(out=v2[:], in0=v1[:], scalar1=float(M))
            nv_o = sb.tile([1, 1], dt.int32)
            nc.vector.tensor_copy(out=nv_o[:], in_=v2[:])
            nc.sync.dma_start(out=new_n_valid.ap()[None, :], in_=nv_o[:])

    nc.finalize()
    return nc


_IN_NAMES = ("activations", "mask", "n_valid", "index")
_OUT_NAMES = ("new_cache", "new_n_valid", "new_index")


def get_nc():
    if "nc" not in _STATE:
        _STATE["nc"] = _build_nc()
    return _STATE["nc"]


def _get_sharded_fn(nc, n_cores):
    """jit(shard_map(bass_exec)) with the output buffers passed in as donated
    inputs — adapted from concourse.bass2jax.run_bass_via_pjrt, except the
    donated buffers carry caller-supplied initial data instead of zeros (the
    NEFF only writes the scattered rows; the rest of new_cache must keep the
    input cache contents)."""
    key = ("fn", n_cores)
    if key in _STATE:
        return _STATE[key]

    import jax
    from jax.experimental.shard_map import shard_map
    from jax.sharding import Mesh, PartitionSpec
    import concourse.mybir as mybir
    from concourse.bass2jax import _bass_exec_p, install_neuronx_cc_hook

    install_neuronx_cc_hook()

    in_names = []
    out_names = []
    out_avals = []
    for alloc in nc.m.functions[0].allocations:
        if not isinstance(alloc, mybir.MemoryLocationSet):
            continue
        name = alloc.memorylocations[0].name
        if alloc.kind == "ExternalInput":
            in_names.append(name)
        elif alloc.kind == "ExternalOutput":
            out_names.append(name)
            out_avals.append(
                jax.core.ShapedArray(tuple(alloc.tensor_shape),
                                     mybir.dt.np(alloc.dtype))
            )
    n_params = len(in_names)
    n_outs = len(out_avals)
    all_in_names = in_names + out_names
    donate = tuple(range(n_params, n_params + n_outs))

    def _body(*args):
        outs = _bass_exec_p.bind(
            *args,
            out_avals=tuple(out_avals),
            in_names=tuple(all_in_names),
            out_names=tuple(out_names),
            lowering_input_output_aliases=(),
            sim_require_finite=True,
            sim_require_nnan=True,
            nc=nc,
        )
        return tuple(outs)

    devices = jax.devices()[:n_cores]
    assert len(devices) == n_cores, f"need {n_cores} devices, have {len(jax.devices())}"
    mesh = Mesh(np.asarray(devices), ("core",))
    spec = PartitionSpec("core")
    sharded = jax.jit(
        shard_map(
            _body, mesh=mesh,
            in_specs=(spec,) * (n_params + n_outs),
            out_specs=(spec,) * n_outs,
            check_rep=False,
        ),
        donate_argnums=donate,
        keep_unused=True,
    )
    _STATE[key] = (sharded, tuple(in_names), tuple(out_names))
    return _STATE[key]


def run_on_cores(global_ins, global_init_outs, n_cores=D):
    """Run the SPMD kernel. `global_ins`/`global_init_outs` are dicts of
    device-count-concatenated arrays (axis 0 splits across cores). Returns a
    dict of global output arrays."""
    nc = get_nc()
    sharded, in_names, out_names = _get_sharded_fn(nc, n_cores)
    args = [np.ascontiguousarray(global_ins[n]) for n in in_names]
    args += [np.ascontiguousarray(global_init_outs[n]) for n in out_names]
    out_arrs = sharded(*args)
    return {n: np.asarray(a) for n, a in zip(out_names, out_arrs)}


def kernel(cache, activations, mask, n_valid, index):
    cache = np.asarray(cache, dtype=np.float32)
    activations = np.asarray(activations, dtype=np.float32)
    mask_u8 = np.asarray(mask).astype(np.uint8)
    n_valid = np.asarray(n_valid, dtype=np.int32)
    index = np.asarray(index, dtype=np.int32)

    global_ins = {
        "activations": activations.reshape(D * T, DIM),
        "mask": mask_u8.reshape(D * T),
        "n_valid": n_valid.reshape(D),
        "index": index.reshape(D),
    }
    global_init_outs = {
        # donated buffer: the input cache — the kernel overwrites only the
        # scattered ring slots
        "new_cache": cache.reshape(D * M, DIM),
        "new_n_valid": np.zeros(D, np.int32),
        "new_index": np.zeros(D, np.int32),
    }
    outs = run_on_cores(global_ins, global_init_outs)
    new_cache = outs["new_cache"].reshape(D, M, DIM)
    new_n_valid = outs["new_n_valid"].reshape(D).astype(np.int32)
    new_index = outs["new_index"].reshape(D).astype(np.int32)
    return new_cache, new_n_valid, new_index


# revision 10
# speedup vs baseline: 1.4003x; 1.0349x over previous
# kernel.py — Trainium2 Bass kernel for nn_ActivationBuffer (ring-buffer masked
# scatter of activations into a cache, data-parallel over 8 NeuronCores).
#
# Self-contained: only imports installed packages (numpy/jax/concourse).
#
# Semantics reproduced per row d (see the reference module):
#   offsets  = cumsum(mask) - 1
#   acc      = offsets[-1] + 1
#   new_n_valid = min(n_valid + acc, M)
#   new_index   = (index + acc) % M
#   new_cache   = cache;  for each kept token j (in order):
#                   new_cache[(index + j) % M] = acts[t_j]
#                 and if mask[0] == 0: new_cache[(index - 1) % M] = 0
#   (the zeroed slot comes from the reference's .at[i].set(0) over slots
#    touched by leading masked-out tokens, whose offset is -1)
#
# Strategy: the kept tokens' destinations are data-dependent, so the kernel
# computes the per-token destination table on-device (mask cumsum via a
# free-axis scan + a triangular matmul across partitions), then uses one
# indirect DMA gather (loads only kept activation rows; masked rows are
# skipped via the bounds check) and one indirect DMA scatter (writes them to
# their ring slots). All other cache slots are preserved by donating the
# input cache shard as the initial contents of the output buffer — the same
# donated-buffer mechanism the stock axon runner uses with zero buffers.

import numpy as np

D, T, M, DIM = 8, 4096, 32768, 768
P = 128           # SBUF partitions
F = T // P        # 32 tokens per partition row

_STATE = {}


def _build_nc():
    from concourse import bacc, bass, mybir, tile
    from concourse.masks import make_upper_triangular

    dt = mybir.dt
    A = mybir.AluOpType

    nc = bacc.Bacc(
        "TRN2",
        target_bir_lowering=False,
        debug=False,
        enable_partition_id=False,
        enable_asserts=False,
    )

    acts = nc.dram_tensor("activations", [T, DIM], dt.float32, kind="ExternalInput")
    mask = nc.dram_tensor("mask", [T], dt.uint8, kind="ExternalInput")
    n_valid = nc.dram_tensor("n_valid", [1], dt.int32, kind="ExternalInput")
    index = nc.dram_tensor("index", [1], dt.int32, kind="ExternalInput")
    new_cache = nc.dram_tensor("new_cache", [M, DIM], dt.float32, kind="ExternalOutput")
    new_n_valid = nc.dram_tensor("new_n_valid", [1], dt.int32, kind="ExternalOutput")
    new_index = nc.dram_tensor("new_index", [1], dt.int32, kind="ExternalOutput")

    with tile.TileContext(nc) as tc:
        with (
            tc.tile_pool(name="sb", bufs=1) as sb,
            tc.tile_pool(name="ps", bufs=1, space="PSUM") as ps,
        ):
            # ---- tiny metadata loads FIRST (they gate the destination-table
            # chain; they must not queue behind the multi-MB activation
            # loads): mask on the SP ring, index/n_valid lead the ACT ring
            m_u8 = sb.tile([P, F], dt.uint8)
            nc.sync.dma_start(out=m_u8[:], in_=mask.ap().rearrange("(p f) -> p f", p=P))
            idx_i = sb.tile([1, 1], dt.int32)
            nc.scalar.dma_start(out=idx_i[:], in_=index.ap()[None, :])
            nv_i = sb.tile([1, 1], dt.int32)
            nc.scalar.dma_start(out=nv_i[:], in_=n_valid.ap()[None, :])

            # ---- activation loads, split across the two HWDGE rings: chunk
            # 0 leads on the (otherwise idle) ACT ring since the first
            # scatters need it
            LC = 8  # load-chunk: tokens per partition per regular DMA
            acts_sb = sb.tile([P, F * DIM], dt.float32)
            acts_pf = acts.ap().rearrange("(p j) d -> p (j d)", p=P)
            for c, eng_name in ((0, "scalar"), (1, "sync"), (2, "scalar"),
                                (3, "sync")):
                es = slice(c * LC * DIM, (c + 1) * LC * DIM)
                getattr(nc, eng_name).dma_start(out=acts_sb[:, es],
                                                in_=acts_pf[:, es])

            m_f = sb.tile([P, F], dt.float32)
            nc.vector.tensor_copy(out=m_f[:], in_=m_u8[:])
            idx_f = sb.tile([1, 1], dt.float32)
            nc.vector.tensor_copy(out=idx_f[:], in_=idx_i[:])
            nv_f = sb.tile([1, 1], dt.float32)
            nc.vector.tensor_copy(out=nv_f[:], in_=nv_i[:])

            # ---- intra-row inclusive cumsum of the mask along the free axis
            zeros_f = sb.tile([P, F], dt.float32)
            nc.vector.memset(zeros_f[:], 0.0)
            C = sb.tile([P, F], dt.float32)
            nc.vector.tensor_tensor_scan(
                out=C[:], data0=m_f[:], data1=zeros_f[:],
                initial=0.0, op0=A.add, op1=A.add,
            )

            # ---- constants for the cross-partition pieces
            stri = sb.tile([P, P], dt.float32)
            make_upper_triangular(nc, stri[:], val=1.0, diag=False)  # stri[k,p]=1 iff k<p
            ones_col = sb.tile([P, 1], dt.float32)
            nc.vector.memset(ones_col[:], 1.0)
            ones_row = sb.tile([1, P], dt.float32)
            nc.vector.memset(ones_row[:], 1.0)

            # ---- E[p] = sum_{k<p} rowtotal[k];  I[p] = index (broadcast)
            e_ps = ps.tile([P, 1], dt.float32)
            nc.tensor.matmul(out=e_ps[:], lhsT=stri[:], rhs=C[:, F - 1:F],
                             start=True, stop=True)
            i_ps = ps.tile([P, 1], dt.float32)
            nc.tensor.matmul(out=i_ps[:], lhsT=ones_row[:], rhs=idx_f[:],
                             start=True, stop=True)
            e_sb = sb.tile([P, 1], dt.float32)
            nc.vector.tensor_copy(out=e_sb[:], in_=e_ps[:])
            B = sb.tile([P, 1], dt.float32)  # index + E[p]
            nc.vector.tensor_tensor(out=B[:], in0=e_sb[:], in1=i_ps[:], op=A.add)

            # ---- acc = total kept tokens, landed on partition 0
            acc_ps = ps.tile([1, 1], dt.float32)
            nc.tensor.matmul(out=acc_ps[:], lhsT=ones_col[:], rhs=C[:, F - 1:F],
                             start=True, stop=True)
            acc = sb.tile([1, 1], dt.float32)
            nc.vector.tensor_copy(out=acc[:], in_=acc_ps[:])

            # ---- leading-masked-token edge case: zero slot (index-1) mod M
            zrow = sb.tile([2, DIM], dt.float32)
            nc.vector.memset(zrow[:], 0.0)
            zoff = sb.tile([2, 1], dt.int32)
            nc.vector.memset(zoff[:], M)
            zt = sb.tile([1, 1], dt.float32)
            nc.vector.tensor_scalar_add(out=zt[:], in0=idx_f[:], scalar1=-1.0)
            zneg = sb.tile([1, 1], dt.float32)
            nc.vector.tensor_scalar(out=zneg[:], in0=zt[:], scalar1=0.0,
                                    scalar2=None, op0=A.is_lt)
            zm = sb.tile([1, 1], dt.float32)
            nc.vector.tensor_scalar_mul(out=zm[:], in0=zneg[:], scalar1=float(M))
            zw = sb.tile([1, 1], dt.float32)
            nc.vector.tensor_tensor(out=zw[:], in0=zt[:], in1=zm[:], op=A.add)
            za = sb.tile([1, 1], dt.float32)
            nc.vector.tensor_scalar_add(out=za[:], in0=zw[:], scalar1=-float(M))
            zb = sb.tile([1, 1], dt.float32)
            nc.vector.tensor_tensor(out=zb[:], in0=za[:], in1=m_f[0:1, 0:1], op=A.mult)
            zc = sb.tile([1, 1], dt.float32)
            nc.vector.tensor_tensor(out=zc[:], in0=za[:], in1=zb[:], op=A.subtract)
            zd = sb.tile([1, 1], dt.float32)
            nc.vector.tensor_scalar_add(out=zd[:], in0=zc[:], scalar1=float(M))
            nc.vector.tensor_copy(out=zoff[0:1, 0:1], in_=zd[:])
            nc.gpsimd.indirect_dma_start(
                out=new_cache.ap(),
                out_offset=bass.IndirectOffsetOnAxis(ap=zoff[:, 0:1], axis=0),
                in_=zrow[:], in_offset=None,
                bounds_check=M - 1, oob_is_err=False,
            )

            # ---- scalar outputs: new_index = (index+acc) mod M,
            # ----                 new_n_valid = min(n_valid+acc, M)
            s1 = sb.tile([1, 1], dt.float32)
            nc.vector.tensor_tensor(out=s1[:], in0=idx_f[:], in1=acc[:], op=A.add)
            s2 = sb.tile([1, 1], dt.float32)
            nc.vector.tensor_scalar(out=s2[:], in0=s1[:], scalar1=float(M),
                                    scalar2=None, op0=A.is_ge)
            s3 = sb.tile([1, 1], dt.float32)
            nc.vector.tensor_scalar_mul(out=s3[:], in0=s2[:], scalar1=float(M))
            s4 = sb.tile([1, 1], dt.float32)
            nc.vector.tensor_tensor(out=s4[:], in0=s1[:], in1=s3[:], op=A.subtract)
            ni_o = sb.tile([1, 1], dt.int32)
            nc.vector.tensor_copy(out=ni_o[:], in_=s4[:])
            nc.sync.dma_start(out=new_index.ap()[None, :], in_=ni_o[:])

            v1 = sb.tile([1, 1], dt.float32)
            nc.vector.tensor_tensor(out=v1[:], in0=nv_f[:], in1=acc[:], op=A.add)
            v2 = sb.tile([1, 1], dt.float32)
            nc.vector.tensor_scalar_min(out=v2[:], in0=v1[:], scalar1=float(M))
            nv_o = sb.tile([1, 1], dt.int32)
            nc.vector.tensor_copy(out=nv_o[:], in_=v2[:])
            nc.sync.dma_start(out=new_n_valid.ap()[None, :], in_=nv_o[:])

            # ---- destination table: kept -> (index + cumsum - 1) mod M, else M
            # draw = (C - 1) + (index + E[p]), fused
            draw = sb.tile([P, F], dt.float32)
            nc.vector.scalar_tensor_tensor(out=draw[:], in0=C[:], scalar=-1.0,
                                           in1=B[:].to_broadcast([P, F]),
                                           op0=A.add, op1=A.add)
            # gm = (draw >= M) * M, fused
            gm = sb.tile([P, F], dt.float32)
            nc.vector.tensor_scalar(out=gm[:], in0=draw[:], scalar1=float(M),
                                    scalar2=float(M), op0=A.is_ge, op1=A.mult)
            dw = sb.tile([P, F], dt.float32)
            nc.vector.tensor_tensor(out=dw[:], in0=draw[:], in1=gm[:], op=A.subtract)
            # dest = (dw - M) * m + M  -> masked-out lanes get sentinel M
            t2 = sb.tile([P, F], dt.float32)
            nc.vector.scalar_tensor_tensor(out=t2[:], in0=dw[:],
                                           scalar=-float(M), in1=m_f[:],
                                           op0=A.add, op1=A.mult)
            dest_f = sb.tile([P, F], dt.float32)
            nc.vector.tensor_scalar_add(out=dest_f[:], in0=t2[:], scalar1=float(M))
            dest = sb.tile([P, F], dt.int32)
            nc.vector.tensor_copy(out=dest[:], in_=dest_f[:])

            # ---- load all activations into SBUF with fast regular DMAs
            # (token t = p*F + j lives at partition p, free chunk j), then
            # scatter kept rows to their ring slots. The HW indirect DMA
            # consumes ONE offset per partition (each feeding that
            # partition's contiguous block), so each scatter op moves one
            # 768-float row per partition using a [P, 1] slice of the dest
            # table; sentinel entries are skipped via the bounds check.
            scatter_insts = []
            for j in range(F):
                si = nc.gpsimd.indirect_dma_start(
                    out=new_cache.ap(),
                    out_offset=bass.IndirectOffsetOnAxis(ap=dest[:, j:j + 1], axis=0),
                    in_=acts_sb[:, j * DIM:(j + 1) * DIM], in_offset=None,
                    bounds_check=M - 1, oob_is_err=False,
                )
                scatter_insts.append(si)
            # The scatters all declare a write of the full new_cache AP, so
            # Tile chains them WAW (each waits for the previous one's DMA
            # completion, ~2us/op). Their actual row sets are disjoint (kept
            # tokens map to distinct ring slots; sentinels are skipped), so
            # drop the inter-scatter edges and let them issue back-to-back.
            _raw = [s.ins for s in scatter_insts]
            _names = {r.name for r in _raw}
            for r in _raw:
                for dep in list(r.sync_dependency_names()) + \
                        list(r.nosync_dependency_names()):
                    if dep in _names:
                        r.remove_dependency(dep)

    nc.finalize()
    return nc


_IN_NAMES = ("activations", "mask", "n_valid", "index")
_OUT_NAMES = ("new_cache", "new_n_valid", "new_index")


def get_nc():
    if "nc" not in _STATE:
        _STATE["nc"] = _build_nc()
    return _STATE["nc"]


def _get_sharded_fn(nc, n_cores):
    """jit(shard_map(bass_exec)) with the output buffers passed in as donated
    inputs — adapted from concourse.bass2jax.run_bass_via_pjrt, except the
    donated buffers carry caller-supplied initial data instead of zeros (the
    NEFF only writes the scattered rows; the rest of new_cache must keep the
    input cache contents)."""
    key = ("fn", n_cores)
    if key in _STATE:
        return _STATE[key]

    import jax
    from jax.experimental.shard_map import shard_map
    from jax.sharding import Mesh, PartitionSpec
    import concourse.mybir as mybir
    from concourse.bass2jax import _bass_exec_p, install_neuronx_cc_hook

    install_neuronx_cc_hook()

    in_names = []
    out_names = []
    out_avals = []
    for alloc in nc.m.functions[0].allocations:
        if not isinstance(alloc, mybir.MemoryLocationSet):
            continue
        name = alloc.memorylocations[0].name
        if alloc.kind == "ExternalInput":
            in_names.append(name)
        elif alloc.kind == "ExternalOutput":
            out_names.append(name)
            out_avals.append(
                jax.core.ShapedArray(tuple(alloc.tensor_shape),
                                     mybir.dt.np(alloc.dtype))
            )
    n_params = len(in_names)
    n_outs = len(out_avals)
    all_in_names = in_names + out_names
    donate = tuple(range(n_params, n_params + n_outs))

    def _body(*args):
        outs = _bass_exec_p.bind(
            *args,
            out_avals=tuple(out_avals),
            in_names=tuple(all_in_names),
            out_names=tuple(out_names),
            lowering_input_output_aliases=(),
            sim_require_finite=True,
            sim_require_nnan=True,
            nc=nc,
        )
        return tuple(outs)

    devices = jax.devices()[:n_cores]
    assert len(devices) == n_cores, f"need {n_cores} devices, have {len(jax.devices())}"
    mesh = Mesh(np.asarray(devices), ("core",))
    spec = PartitionSpec("core")
    sharded = jax.jit(
        shard_map(
            _body, mesh=mesh,
            in_specs=(spec,) * (n_params + n_outs),
            out_specs=(spec,) * n_outs,
            check_rep=False,
        ),
        donate_argnums=donate,
        keep_unused=True,
    )
    _STATE[key] = (sharded, tuple(in_names), tuple(out_names))
    return _STATE[key]


def run_on_cores(global_ins, global_init_outs, n_cores=D):
    """Run the SPMD kernel. `global_ins`/`global_init_outs` are dicts of
    device-count-concatenated arrays (axis 0 splits across cores). Returns a
    dict of global output arrays."""
    nc = get_nc()
    sharded, in_names, out_names = _get_sharded_fn(nc, n_cores)
    args = [np.ascontiguousarray(global_ins[n]) for n in in_names]
    args += [np.ascontiguousarray(global_init_outs[n]) for n in out_names]
    out_arrs = sharded(*args)
    return {n: np.asarray(a) for n, a in zip(out_names, out_arrs)}


def kernel(cache, activations, mask, n_valid, index):
    cache = np.asarray(cache, dtype=np.float32)
    activations = np.asarray(activations, dtype=np.float32)
    mask_u8 = np.asarray(mask).astype(np.uint8)
    n_valid = np.asarray(n_valid, dtype=np.int32)
    index = np.asarray(index, dtype=np.int32)

    global_ins = {
        "activations": activations.reshape(D * T, DIM),
        "mask": mask_u8.reshape(D * T),
        "n_valid": n_valid.reshape(D),
        "index": index.reshape(D),
    }
    global_init_outs = {
        # donated buffer: the input cache — the kernel overwrites only the
        # scattered ring slots
        "new_cache": cache.reshape(D * M, DIM),
        "new_n_valid": np.zeros(D, np.int32),
        "new_index": np.zeros(D, np.int32),
    }
    outs = run_on_cores(global_ins, global_init_outs)
    new_cache = outs["new_cache"].reshape(D, M, DIM)
    new_n_valid = outs["new_n_valid"].reshape(D).astype(np.int32)
    new_index = outs["new_index"].reshape(D).astype(np.int32)
    return new_cache, new_n_valid, new_index
